# revision 1
# baseline (speedup 1.0000x reference)
"""Trainium2 Bass kernel for ConcatBiInteraction (gnn_message_passing).

Model (per molecule segment b with atoms n, protein rows l):
  hid[n,l,:]  = tanh(att1_Wp.T@prot[b,l] + att1_Wa.T@atom[n] + b1)      (128 hidden)
  Wraw[n,l]   = tanh(hid[n,l,:]@w2 + b2)         (reference W = 5*Wraw)
  Wc[n]       = exp(5*max_l Wraw[n,l]);  aa = Wc / segsum(Wc)
  atom_pool[b]= sum_n aa[n]*atom[n]
  Wp[b,l]     = max_{n in b} Wraw[n,l];  ap = softmax(5*Wp)  (exp(5*x), no max-sub
                needed: |5*Wp|<=5 so exp in [6.7e-3, 148])
  prot_pool[b]= sum_l ap[l]*prot[b,l]
  out         = MLP(concat(atom_pool, prot_pool))

Sharding: molecules (and their contiguous atom ranges + protein rows) are
partitioned 4-per-core across 8 cores; no cross-core communication.

On-chip layout: hidden dim k=128 lives on SBUF partitions.  Per molecule,
paT[k,l] is one matmul; per atom the (+ab, tanh) is a DVE tensor_scalar add
(per-partition scalar ab[k]) followed by one big batched ACT tanh; the w2
dot-product is a matmul with lhsT = w2 placed in column (slot%32) of a
zero-padded [128,32] block, accumulating atoms into distinct partitions of
one PSUM bank (col-tiling via tile_position).  All segment reductions become
small matmuls against host-built masks.
"""

import numpy as np

import concourse.bass as bass
import concourse.tile as tile
from concourse import bacc, mybir
from concourse.bass_utils import run_bass_kernel_spmd

FP = mybir.dt.float32
BF = mybir.dt.bfloat16
AF = mybir.ActivationFunctionType

B, L, P, A = 32, 512, 128, 128
N = 1024
H1, H2 = 512, 256
NCORES = 8
MPC = B // NCORES          # molecules per core
ABATCH = 16                # atoms per tanh batch
GP_PER_BATCH = 6           # per-atom adds offloaded DVE -> GPSIMD per batch
PAD_NEG = -10.0            # pushes padded-slot scores below any tanh output

_cache: dict = {}


def _build(cap: int, padded: bool):
    """Build + compile the SPMD program for `cap` atom slots per molecule."""
    slots = MPC * cap                 # atom slots per core
    n_blocks = slots // 128           # 128-slot blocks per core
    assert cap % 32 == 0 and slots % 128 == 0

    nc = bacc.Bacc("TRN2", target_bir_lowering=False, debug=False)

    def din(name, shape, dt=FP):
        return nc.dram_tensor(name, list(shape), dt, kind="ExternalInput").ap()

    prot_T = din("prot_T", [MPC, P, L], BF)    # protSeq[m].T
    prot_N = din("prot_N", [MPC, L, P], BF)    # protSeq[m] natural
    atom_N = din("atom_N", [slots, A])         # padded atoms, slot order
    atom_T = din("atom_T", [A, slots])
    att1_Wp = din("att1_Wp", [P, 128], BF)
    att1_Wa = din("att1_Wa", [A, 128])
    b1 = din("b1", [128, 1])
    w2b = din("w2b", [128, 32 * 32], BF)       # block i = w2 in column i
    b2vec = din("b2vec", [128, 1])
    seg_m = din("seg_m", [n_blocks, 128, MPC])     # slot->mol indicator (0 for pad)
    pad_add = din("pad_add", [n_blocks, 128, 1])   # 0 real / PAD_NEG pad
    ident = din("ident", [128, 128])
    d1_W = din("d1_W", [P + A, H1])
    d1_b4 = din("d1_b4", [128, H1 // 128])
    d2_W = din("d2_W", [H1, H2])
    d2_b2 = din("d2_b2", [128, H2 // 128])
    oW = din("oW", [H2, 1])
    ob = din("ob", [1, 1])
    ones_c = din("ones_c", [128, 1], BF)
    ones_r = din("ones_r", [1, 128])
    out_d = nc.dram_tensor("out", [MPC, 1], FP, kind="ExternalOutput").ap()

    NJ1 = H1 // 128   # 4 j-blocks of hidden1
    NJ2 = H2 // 128   # 2 i-blocks of hidden2

    from contextlib import ExitStack
    with tile.TileContext(nc) as tc, ExitStack() as ctx:
        cpool = ctx.enter_context(tc.tile_pool(name="consts", bufs=1))
        hpool = ctx.enter_context(tc.tile_pool(name="hid", bufs=2))
        htpool = ctx.enter_context(tc.tile_pool(name="ht", bufs=3))
        spool = ctx.enter_context(tc.tile_pool(name="smallsb", bufs=2))
        pp_pa = ctx.enter_context(tc.tile_pool(name="pa", bufs=2, space="PSUM"))
        pp_wb = ctx.enter_context(
            tc.tile_pool(name="wb", bufs=min(2, n_blocks), space="PSUM"))
        pp_mm = ctx.enter_context(tc.tile_pool(name="mm", bufs=2, space="PSUM"))
        pp_sc = ctx.enter_context(tc.tile_pool(name="sc", bufs=1, space="PSUM"))
        pp_ap = ctx.enter_context(tc.tile_pool(name="ap", bufs=1, space="PSUM"))

        def load(ap_in, shape, name, dt=FP, eng=None):
            t = cpool.tile(list(shape), dt, tag=name)
            (eng or nc.sync).dma_start(t[:], ap_in)
            return t

        # ---- critical-path loads on the HWDGE (sync) queue ----
        wp_sb = load(att1_Wp[:], [128, 128], "wp", BF)
        wa_sb = load(att1_Wa[:], [128, 128], "wa")
        b1_sb = load(b1[:], [128, 1], "b1")
        atomT_sb = load(atom_T[:], [128, slots], "atomT")
        protT_sb = cpool.tile([128, MPC * L], BF, tag="protT")
        nc.sync.dma_start(protT_sb[:].rearrange("p (m l) -> p m l", m=MPC),
                          prot_T[:].rearrange("m p l -> p m l"))
        w2b_sb = load(w2b[:], [128, 1024], "w2b", BF)
        b2_sb = load(b2vec[:], [128, 1], "b2")

        # ---- bulk loads on the gpsimd queue ----
        gp = nc.sync
        seg_sb = cpool.tile([128, n_blocks * MPC], FP, tag="seg")
        gp.dma_start(seg_sb[:].rearrange("p (b f) -> p b f", b=n_blocks),
                     seg_m[:].rearrange("b p f -> p b f"))
        pad_sb = cpool.tile([128, n_blocks], FP, tag="pad")
        gp.dma_start(pad_sb[:].rearrange("p (b f) -> p b f", b=n_blocks),
                     pad_add[:].rearrange("b p f -> p b f"))
        id_sb = load(ident[:], [128, 128], "ident", FP, gp)
        atomN_sb = cpool.tile([128, n_blocks * A], FP, tag="atomN")
        gp.dma_start(atomN_sb[:].rearrange("p (b f) -> p b f", b=n_blocks),
                     atom_N[:].rearrange("(b p) f -> p b f", b=n_blocks))
        protN_sb = cpool.tile([128, MPC * L], BF, tag="protN")
        gp.dma_start(protN_sb[:].rearrange("p (mc f) -> p mc f", mc=4 * MPC),
                     prot_N[:].rearrange("m (c p) f -> p (m c) f", c=4))
        d1_sb = cpool.tile([128, 2 * H1], FP, tag="d1")
        gp.dma_start(d1_sb[:].rearrange("p (kc f) -> p kc f", kc=2),
                     d1_W[:].rearrange("(kc p) f -> p kc f", kc=2))
        d1b_sb = load(d1_b4[:], [128, NJ1], "d1b", FP, gp)
        d2_sb = cpool.tile([128, 4 * H2], FP, tag="d2")
        gp.dma_start(d2_sb[:].rearrange("p (j f) -> p j f", j=4),
                     d2_W[:].rearrange("(j p) f -> p j f", j=4))
        d2b_sb = load(d2_b2[:], [128, NJ2], "d2b", FP, gp)
        ow_sb = cpool.tile([128, 2], FP, tag="ow")
        gp.dma_start(ow_sb[:].rearrange("p (i u) -> p i u", i=2),
                     oW[:].rearrange("(i p) u -> p i u", i=2))
        ob_sb = load(ob[:], [1, 1], "ob", FP, gp)
        onec_sb = load(ones_c[:], [128, 1], "onec", BF, gp)
        oner_sb = load(ones_r[:], [1, 128], "oner", FP, gp)

        # ---- ab[k, slot] = Wa.T @ atomT + b1 ----
        abT_sb = cpool.tile([128, slots], FP, tag="abT")
        for b in range(n_blocks):
            ps = pp_mm.tile([128, 128], FP, tag="mm")
            nc.tensor.matmul(ps[:], wa_sb[:], atomT_sb[:, b * 128:(b + 1) * 128])
            nc.vector.tensor_scalar_add(abT_sb[:, b * 128:(b + 1) * 128],
                                        ps[:], b1_sb[:, 0:1])

        # ---- paT[k, l] per molecule (bf16: feeds the per-atom DVE adds) ----
        paT_sb = cpool.tile([128, MPC * L], BF, tag="paT")
        for m in range(MPC):
            ps = pp_pa.tile([128, L], FP, tag="pa")
            nc.tensor.matmul(ps[:], wp_sb[:], protT_sb[:, m * L:(m + 1) * L])
            nc.vector.tensor_copy(paT_sb[:, m * L:(m + 1) * L], ps[:])

        # ---- main loop: hid -> tanh -> w2 dot, 128 atoms per W bank ----
        W_sb = cpool.tile([128, n_blocks * L], FP, tag="W")
        Wc_sb = cpool.tile([128, n_blocks], FP, tag="Wc")
        wc4_sb = cpool.tile([128, n_blocks * MPC], FP, tag="wc4")
        sc_ps = pp_sc.tile([1, MPC], FP, tag="sc")
        ap_ps = pp_ap.tile([128, MPC], FP, tag="ap")
        for b in range(n_blocks):
            wb_ps = pp_wb.tile([128, L], FP, tag="wb")
            # ramp the first batches so ACT starts early
            sizes = [4, 4, 8] + [16] * 7 if b == 0 else [16] * 8
            t = 0
            for size in sizes:
                pre = hpool.tile([128, size * L], BF, tag="pre")
                batch = []
                for a in range(size):
                    tt = t + a
                    s = b * 128 + (tt % 4) * 32 + tt // 4   # striped col-groups
                    batch.append(s)
                    m = s // cap
                    nc.vector.tensor_scalar_add(
                        pre[:, a * L:(a + 1) * L],
                        paT_sb[:, m * L:(m + 1) * L], abT_sb[:, s:s + 1])
                ht = htpool.tile([128, size * L], BF, tag="ht")
                nc.scalar.activation(ht[:], pre[:], AF.Tanh)
                for a, s in enumerate(batch):
                    i = s % 32                            # column in group
                    j = (s % 128) // 32                   # col-group
                    nc.tensor.matmul(
                        wb_ps[32 * j:32 * (j + 1), :],
                        w2b_sb[:, i * 32:(i + 1) * 32],
                        ht[:, a * L:(a + 1) * L],
                        start=(i == 0), stop=(i == 31),
                        skip_group_check=True,
                        tile_position=(0, 32 * j))
                t += size

            # Wraw (+pad offset) for this block of 128 slots
            wofs = b * L
            nc.scalar.activation(W_sb[:, wofs:wofs + L], wb_ps[:], AF.Tanh,
                                 bias=b2_sb[:, 0:1])
            if padded:
                nc.vector.tensor_scalar_add(W_sb[:, wofs:wofs + L],
                                            W_sb[:, wofs:wofs + L],
                                            pad_sb[:, b:b + 1])
            # atom-side weight Wc = exp(5 * max_l), masked per molecule
            wc_pre = spool.tile([128, 1], FP, tag="wcpre")
            nc.vector.reduce_max(wc_pre[:], W_sb[:, wofs:wofs + L],
                                 axis=mybir.AxisListType.X)
            nc.scalar.activation(Wc_sb[:, b:b + 1], wc_pre[:], AF.Exp, scale=5.0)
            nc.vector.tensor_mul(
                wc4_sb[:, b * MPC:(b + 1) * MPC],
                seg_sb[:, b * MPC:(b + 1) * MPC],
                Wc_sb[:, b:b + 1].to_broadcast([128, MPC]))
            wc4_bf = spool.tile([128, MPC], BF, tag="wc4bf")
            nc.vector.tensor_copy(wc4_bf[:], wc4_sb[:, b * MPC:(b + 1) * MPC])
            # segment sums of Wc and unnormalized atom_poolT, across blocks
            nc.tensor.matmul(sc_ps[:], onec_sb[:], wc4_bf[:],
                             start=(b == 0), stop=(b == n_blocks - 1))
            nc.tensor.matmul(ap_ps[:], atomN_sb[:, b * A:(b + 1) * A],
                             wc4_sb[:, b * MPC:(b + 1) * MPC],
                             start=(b == 0), stop=(b == n_blocks - 1))

        # ---- Wp: transpose W chunks, segmented max over slots ----
        WpT_sb = cpool.tile([128, 4 * MPC], FP, tag="WpT")   # cols c*MPC+m
        first_piece = {}
        for b in range(n_blocks):
            for c in range(4):
                tp = pp_mm.tile([128, 128], FP, tag="mm")
                nc.tensor.transpose(
                    tp[:], W_sb[:, b * L + c * 128:b * L + (c + 1) * 128], id_sb[:])
                for m in range(MPC):
                    lo = max(m * cap, b * 128) - b * 128
                    hi = min((m + 1) * cap, (b + 1) * 128) - b * 128
                    if lo >= hi:
                        continue
                    col = c * MPC + m
                    if col not in first_piece:
                        first_piece[col] = True
                        nc.vector.reduce_max(WpT_sb[:, col:col + 1],
                                             tp[:, lo:hi],
                                             axis=mybir.AxisListType.X)
                    else:
                        tmp = spool.tile([128, 1], FP, tag="wtmp")
                        nc.vector.reduce_max(tmp[:], tp[:, lo:hi],
                                             axis=mybir.AxisListType.X)
                        nc.vector.tensor_max(WpT_sb[:, col:col + 1],
                                             WpT_sb[:, col:col + 1], tmp[:])

        expW_sb = spool.tile([128, 4 * MPC], BF, tag="expW")
        nc.scalar.activation(expW_sb[:], WpT_sb[:], AF.Exp, scale=5.0)

        # softmax denominators per molecule (cols 4..8 of the joint recip)
        den_ps = pp_mm.tile([1, 4 * MPC], FP, tag="mm")
        nc.tensor.matmul(den_ps[:], onec_sb[:], expW_sb[:])
        nrm = spool.tile([1, 2 * MPC], FP, tag="nrm")
        nc.vector.tensor_copy(nrm[:, 0:MPC], sc_ps[:])
        nc.vector.reduce_sum(nrm[:, MPC:2 * MPC],
                             den_ps[:].rearrange("p (c m) -> p m c", m=MPC),
                             axis=mybir.AxisListType.X)
        rnrm = spool.tile([1, 2 * MPC], FP, tag="rnrm")
        nc.vector.reciprocal(rnrm[:], nrm[:])
        rb_ps = pp_mm.tile([128, 2 * MPC], FP, tag="mm")
        nc.tensor.matmul(rb_ps[:], oner_sb[:], rnrm[:])
        rb_sb = spool.tile([128, 2 * MPC], FP, tag="rb")
        nc.vector.tensor_copy(rb_sb[:], rb_ps[:])

        # normalized pools
        apT_sb = spool.tile([128, MPC], FP, tag="apT")
        nc.vector.tensor_mul(apT_sb[:], ap_ps[:], rb_sb[:, 0:MPC])
        pp_ps = pp_mm.tile([128, MPC], FP, tag="mm")
        for m in range(MPC):
            for c in range(4):
                nc.tensor.matmul(pp_ps[:, m:m + 1],
                                 protN_sb[:, (m * 4 + c) * 128:(m * 4 + c + 1) * 128],
                                 expW_sb[:, c * MPC + m:c * MPC + m + 1],
                                 start=(c == 0), stop=(c == 3))
        ppT_sb = spool.tile([128, MPC], FP, tag="ppT")
        nc.vector.tensor_mul(ppT_sb[:], pp_ps[:], rb_sb[:, MPC:2 * MPC])

        # ---- output MLP (transposed: molecules on the free axis) ----
        h1_sb = spool.tile([128, NJ1 * MPC], FP, tag="h1")
        for j in range(NJ1):
            h1_ps = pp_mm.tile([128, MPC], FP, tag="mm")
            nc.tensor.matmul(h1_ps[:], d1_sb[:, j * 128:(j + 1) * 128],
                             apT_sb[:], start=True, stop=False)
            nc.tensor.matmul(h1_ps[:], d1_sb[:, H1 + j * 128:H1 + (j + 1) * 128],
                             ppT_sb[:], start=False, stop=True)
            nc.scalar.activation(h1_sb[:, j * MPC:(j + 1) * MPC], h1_ps[:],
                                 AF.Relu, bias=d1b_sb[:, j:j + 1])
        h2_sb = spool.tile([128, NJ2 * MPC], FP, tag="h2")
        for i in range(NJ2):
            h2_ps = pp_mm.tile([128, MPC], FP, tag="mm")
            for j in range(NJ1):
                nc.tensor.matmul(h2_ps[:],
                                 d2_sb[:, j * H2 + i * 128:j * H2 + (i + 1) * 128],
                                 h1_sb[:, j * MPC:(j + 1) * MPC],
                                 start=(j == 0), stop=(j == NJ1 - 1))
            nc.scalar.activation(h2_sb[:, i * MPC:(i + 1) * MPC], h2_ps[:],
                                 AF.Relu, bias=d2b_sb[:, i:i + 1])
        o_ps = pp_mm.tile([1, MPC], FP, tag="mm")
        for i in range(NJ2):
            nc.tensor.matmul(o_ps[:], ow_sb[:, i:i + 1],
                             h2_sb[:, i * MPC:(i + 1) * MPC],
                             start=(i == 0), stop=(i == NJ2 - 1))
        o_sb = spool.tile([1, MPC], FP, tag="o")
        nc.scalar.activation(o_sb[:], o_ps[:], AF.Identity, bias=ob_sb[0:1, 0:1])
        nc.sync.dma_start(out_d[:], o_sb[0:1, :])

    nc.compile()
    return nc


def prepare(atom_embed, protSeq_embed, atom_splits,
            att1_W, att1_b, att2_W, att2_b,
            d1_W, d1_b, d2_W, d2_b, out_W, out_b):
    atom_embed = np.ascontiguousarray(atom_embed, dtype=np.float32)
    protSeq_embed = np.ascontiguousarray(protSeq_embed, dtype=np.float32)
    splits = np.asarray(atom_splits).astype(np.int64)
    assert atom_embed.shape == (N, A) and protSeq_embed.shape == (B, L, P)

    counts = np.bincount(splits, minlength=B)
    starts = np.concatenate([[0], np.cumsum(counts)])[:B]
    cap = max(32, int(-(-counts.max() // 32)) * 32)
    slots = MPC * cap
    n_blocks = slots // 128
    padded = bool((counts != cap).any())

    key = (cap, padded)
    if key not in _cache:
        _cache[key] = _build(cap, padded)
    nc = _cache[key]

    import ml_dtypes
    f32 = np.float32
    # shared (per-core-identical) weight tensors
    w2b = np.zeros((128, 32, 32), f32)
    w2b[:, np.arange(32), np.arange(32)] = np.asarray(att2_W, f32)[:, 0:1].repeat(32, 1)
    w2b = w2b.astype(ml_dtypes.bfloat16)
    shared = {
        "att1_Wp": np.ascontiguousarray(att1_W[:P], f32).astype(ml_dtypes.bfloat16),
        "att1_Wa": np.ascontiguousarray(att1_W[P:], f32),
        "b1": np.asarray(att1_b, f32).reshape(128, 1),
        "w2b": w2b.reshape(128, 1024),
        "b2vec": np.full((128, 1), np.asarray(att2_b, f32)[0], f32),
        "ident": np.eye(128, dtype=f32),
        "d1_W": np.ascontiguousarray(d1_W, f32),
        "d1_b4": np.ascontiguousarray(np.asarray(d1_b, f32).reshape(H1 // 128, 128).T),
        "d2_W": np.ascontiguousarray(d2_W, f32),
        "d2_b2": np.ascontiguousarray(np.asarray(d2_b, f32).reshape(H2 // 128, 128).T),
        "oW": np.asarray(out_W, f32).reshape(H2, 1),
        "ob": np.asarray(out_b, f32).reshape(1, 1),
        "ones_c": np.ones((128, 1), ml_dtypes.bfloat16),
        "ones_r": np.ones((1, 128), f32),
    }

    in_maps = []
    for c in range(NCORES):
        gm = range(MPC * c, MPC * (c + 1))
        aN = np.zeros((slots, A), f32)
        seg = np.zeros((n_blocks, 128, MPC), f32)
        pad = np.full((n_blocks, 128, 1), PAD_NEG, f32)
        for lm, g in enumerate(gm):
            cnt = int(counts[g])
            s0 = lm * cap
            aN[s0:s0 + cnt] = atom_embed[starts[g]:starts[g] + cnt]
            sl = np.arange(s0, s0 + cnt)
            seg[sl // 128, sl % 128, lm] = 1.0
            pad[sl // 128, sl % 128, 0] = 0.0
        pmc = protSeq_embed[MPC * c:MPC * (c + 1)]
        in_maps.append({
            **shared,
            "prot_T": np.ascontiguousarray(pmc.transpose(0, 2, 1)).astype(ml_dtypes.bfloat16),
            "prot_N": np.ascontiguousarray(pmc).astype(ml_dtypes.bfloat16),
            "atom_N": aN,
            "atom_T": np.ascontiguousarray(aN.T),
            "seg_m": seg,
            "pad_add": pad,
        })

    return nc, in_maps


def kernel(**inputs):
    nc, in_maps = prepare(**inputs)
    res = run_bass_kernel_spmd(nc, in_maps, list(range(NCORES)))
    return np.concatenate([res.results[c]["out"] for c in range(NCORES)], axis=0)



# revision 2
# speedup vs baseline: 1.3517x; 1.3517x over previous
"""Trainium2 Bass kernel for ConcatBiInteraction — sin-factorized scores.

Math: the score matrix s[n,l] = sum_k w2[k]*tanh(pa[k,l] + ab[k,n]) is the
bottleneck (N*L*K tanh evals).  Approximate tanh(z) ~= sum_q A_q sin(W_q z)
(Q=6, Gaussian-weighted fit, wrms ~7e-5); the sin addition formula turns the
score into 2Q rank-1 products computed as TensorE matmuls:
  s[n,l] ~= sum_q A_q [ sin(W_q pa) cos(W_q ab) + cos(W_q pa) sin(W_q ab) ]
Per frequency the sin/cos pairs are produced by the ACT engine from an
integer-trick range reduction (DVE: y=int32(pa'*W_q/2pi*2^18); t=y&(2^18-1))
because the ACT Sin spline is only valid for |arg| <= ~3.9 rad.  With
pa' = pa + C > 0, t represents (W_q pa' mod 2pi) exactly; the two ACT
instructions sin(S2*t - 5pi/4) and sin(S2*t - 3pi/4) stay inside the valid
window and equal -sin/-cos of the true angle + const phase; all signs and
phases fold into the host-computed per-frequency F-side phase D_q and the
coefficient vectors A_q*w2[k].

Sharding: 4 molecules per core x 8 cores (block-contiguous atoms), no
cross-core communication.  Downstream (Wc/Wp maxes, softmax pools, MLP)
reuses the segment-mask matmul machinery of the direct kernel.
"""

import numpy as np

import concourse.bass as bass
import concourse.tile as tile
from concourse import bacc, mybir
from concourse.bass_utils import run_bass_kernel_spmd

FP = mybir.dt.float32
BF = mybir.dt.bfloat16
I32 = mybir.dt.int32
AF = mybir.ActivationFunctionType
ALU = mybir.AluOpType

B, L, P, A = 32, 512, 128, 128
N = 1024
H1, H2 = 512, 256
NCORES = 8
MPC = B // NCORES          # molecules per core
PAD_NEG = -10.0

# sin-basis fit of tanh on [-6.5, 6.5], Gaussian-weighted (fit_sin.py, Q=6)
OMEGAS = [0.3118, 0.977, 1.7192, 2.5326, 3.3921, 4.5457]
ACOEF = [1.24009, 0.31808, 0.10507, 0.0316, 0.00877, 0.00244]
QF = len(OMEGAS)
BITS = 18
PERIOD = 1 << BITS
MASK = PERIOD - 1
S2 = float(2 * np.pi / PERIOD)
C_PA = 8.0
C_AB = 8.0
B5 = float(-5 * np.pi / 4)
B3 = float(-3 * np.pi / 4)
SCQ = [float(np.float32(om / (2 * np.pi) * PERIOD)) for om in OMEGAS]
DQ = [float(np.float32(np.mod(-om * (C_PA + C_AB) + np.pi / 2, 2 * np.pi)
                       / (2 * np.pi) * PERIOD)) for om in OMEGAS]

_cache: dict = {}


def _build(padded: bool):
    """cap == 32 fast path: 128 atom slots per core, one slot block."""
    nc = bacc.Bacc("TRN2", target_bir_lowering=False, debug=False)

    def din(name, shape, dt=FP):
        return nc.dram_tensor(name, list(shape), dt, kind="ExternalInput").ap()

    prot_T = din("prot_T", [MPC, P, L], BF)
    prot_N = din("prot_N", [MPC, L, P], BF)
    atom_N = din("atom_N", [128, A])
    atom_T = din("atom_T", [A, 128])
    att1_Wp = din("att1_Wp", [P, 128], BF)
    att1_Wa = din("att1_Wa", [A, 128])
    b1 = din("b1", [128, 1])
    fcs = din("fcs", [128, QF])            # A_q * w2[k]
    b2vec = din("b2vec", [128, 1])
    biasE = din("biasE", [128, 2])         # col0=-5pi/4  col1=-3pi/4
    seg_m = din("seg_m", [128, MPC])       # slot->mol indicator (0 for pad)
    pad_add = din("pad_add", [128, 1])     # 0 real / PAD_NEG pad
    ident = din("ident", [128, 128])
    d1_W = din("d1_W", [P + A, H1])
    d1_b4 = din("d1_b4", [128, H1 // 128])
    d2_W = din("d2_W", [H1, H2])
    d2_b2 = din("d2_b2", [128, H2 // 128])
    oW = din("oW", [H2, 1])
    ob = din("ob", [1, 1])
    ones_c = din("ones_c", [128, 1], BF)
    ones_r = din("ones_r", [1, 128])
    out_d = nc.dram_tensor("out", [MPC, 1], FP, kind="ExternalOutput").ap()

    NJ1 = H1 // 128
    NJ2 = H2 // 128
    CL = MPC * L              # 2048 score columns (m, l)

    from contextlib import ExitStack
    with tile.TileContext(nc) as tc, ExitStack() as ctx:
        cpool = ctx.enter_context(tc.tile_pool(name="consts", bufs=1))
        tpool = ctx.enter_context(tc.tile_pool(name="tred", bufs=2))
        epool = ctx.enter_context(tc.tile_pool(name="etile", bufs=4))
        spool = ctx.enter_context(tc.tile_pool(name="smallsb", bufs=2))
        pp_s = ctx.enter_context(tc.tile_pool(name="sacc", bufs=1, space="PSUM"))
        pp_pa = ctx.enter_context(tc.tile_pool(name="pa", bufs=2, space="PSUM"))
        pp_mm = ctx.enter_context(tc.tile_pool(name="mm", bufs=2, space="PSUM"))

        def load(ap_in, shape, name, dt=FP, eng=None):
            t = cpool.tile(list(shape), dt, tag=name)
            (eng or nc.sync).dma_start(t[:], ap_in)
            return t

        # ---- critical-path loads ----
        wa_sb = load(att1_Wa[:], [128, 128], "wa")
        atomT_sb = load(atom_T[:], [128, 128], "atomT")
        b1_sb = load(b1[:], [128, 1], "b1")
        fcs_sb = load(fcs[:], [128, QF], "fcs")
        be_sb = load(biasE[:], [128, 2], "biasE")
        wp_sb = load(att1_Wp[:], [128, 128], "wp", BF)
        protT_sb = cpool.tile([128, CL], BF, tag="protT")
        nc.sync.dma_start(protT_sb[:].rearrange("p (m l) -> p m l", m=MPC),
                          prot_T[:].rearrange("m p l -> p m l"))
        b2_sb = load(b2vec[:], [128, 1], "b2")

        # ---- bulk loads ----
        gp = nc.sync
        seg_sb = load(seg_m[:], [128, MPC], "seg", FP, gp)
        pad_sb = load(pad_add[:], [128, 1], "pad", FP, gp)
        id_sb = load(ident[:], [128, 128], "ident", FP, gp)
        atomN_sb = load(atom_N[:], [128, A], "atomN", FP, gp)
        protN_sb = cpool.tile([128, CL], BF, tag="protN")
        gp.dma_start(protN_sb[:].rearrange("p (mc f) -> p mc f", mc=4 * MPC),
                     prot_N[:].rearrange("m (c p) f -> p (m c) f", c=4))
        d1_sb = cpool.tile([128, 2 * H1], FP, tag="d1")
        gp.dma_start(d1_sb[:].rearrange("p (kc f) -> p kc f", kc=2),
                     d1_W[:].rearrange("(kc p) f -> p kc f", kc=2))
        d1b_sb = load(d1_b4[:], [128, NJ1], "d1b", FP, gp)
        d2_sb = cpool.tile([128, 4 * H2], FP, tag="d2")
        gp.dma_start(d2_sb[:].rearrange("p (j f) -> p j f", j=4),
                     d2_W[:].rearrange("(j p) f -> p j f", j=4))
        d2b_sb = load(d2_b2[:], [128, NJ2], "d2b", FP, gp)
        ow_sb = cpool.tile([128, 2], FP, tag="ow")
        gp.dma_start(ow_sb[:].rearrange("p (i u) -> p i u", i=2),
                     oW[:].rearrange("(i p) u -> p i u", i=2))
        ob_sb = load(ob[:], [1, 1], "ob", FP, gp)
        onec_sb = load(ones_c[:], [128, 1], "onec", BF, gp)
        oner_sb = load(ones_r[:], [1, 128], "oner", FP, gp)

        # ---- ab' = Wa.T @ atomT + b1 + C_AB  (fp32) ----
        ab_ps = pp_mm.tile([128, 128], FP, tag="mm")
        nc.tensor.matmul(ab_ps[:], wa_sb[:], atomT_sb[:])
        abp_sb = cpool.tile([128, 128], FP, tag="abp")
        nc.vector.tensor_scalar(abp_sb[:], ab_ps[:], b1_sb[:, 0:1], C_AB,
                                ALU.add, ALU.add)

        # ---- F side: per-q reduced angles, sin/cos, coefficient fold ----
        abt_i = cpool.tile([128, QF * 128], I32, tag="abt")
        for q in range(QF):
            nc.vector.tensor_scalar(abt_i[:, q * 128:(q + 1) * 128], abp_sb[:],
                                    SCQ[q], DQ[q], ALU.mult, ALU.add)
        abt_r = cpool.tile([128, QF * 128], I32, tag="abtr")
        nc.vector.tensor_scalar(abt_r[:], abt_i[:], MASK, None, ALU.bitwise_and)
        f1_sb = cpool.tile([128, QF * 128], FP, tag="f1")   # cos-partner of E1
        nc.scalar.activation(f1_sb[:], abt_r[:], AF.Sin, scale=S2,
                             bias=be_sb[:, 1:2])
        f2_sb = cpool.tile([128, QF * 128], FP, tag="f2")   # sin-partner of E2
        nc.scalar.activation(f2_sb[:], abt_r[:], AF.Sin, scale=S2,
                             bias=be_sb[:, 0:1])
        fc1_sb = cpool.tile([128, QF * 128], BF, tag="fc1")
        fc2_sb = cpool.tile([128, QF * 128], BF, tag="fc2")
        for q in range(QF):
            nc.vector.tensor_scalar(fc1_sb[:, q * 128:(q + 1) * 128],
                                    f1_sb[:, q * 128:(q + 1) * 128],
                                    fcs_sb[:, q:q + 1], None, ALU.mult)
            nc.vector.tensor_scalar(fc2_sb[:, q * 128:(q + 1) * 128],
                                    f2_sb[:, q * 128:(q + 1) * 128],
                                    fcs_sb[:, q:q + 1], None, ALU.mult)

        # ---- pa' = Wp.T @ protT + C_PA  (fp32, [128, 2048]) ----
        pap_sb = cpool.tile([128, CL], FP, tag="pap")
        for m in range(MPC):
            ps = pp_pa.tile([128, L], FP, tag="pa")
            nc.tensor.matmul(ps[:], wp_sb[:], protT_sb[:, m * L:(m + 1) * L])
            nc.vector.tensor_scalar(pap_sb[:, m * L:(m + 1) * L], ps[:],
                                    C_PA, None, ALU.add)

        # ---- main loop: per-frequency rank-2 accumulation ----
        s_ps = [pp_s.tile([128, L], FP, tag=f"s{c}", name=f"s_ps{c}")
                for c in range(MPC)]
        for q in range(QF):
            te_i = tpool.tile([128, CL], I32, tag="tei")
            nc.vector.tensor_scalar(te_i[:], pap_sb[:], SCQ[q], None, ALU.mult)
            te_r = tpool.tile([128, CL], I32, tag="ter")
            nc.vector.tensor_scalar(te_r[:], te_i[:], MASK, None, ALU.bitwise_and)
            e1 = epool.tile([128, CL], BF, tag="e1")
            nc.scalar.activation(e1[:], te_r[:], AF.Sin, scale=S2,
                                 bias=be_sb[:, 0:1])
            e2 = epool.tile([128, CL], BF, tag="e2")
            nc.scalar.activation(e2[:], te_r[:], AF.Sin, scale=S2,
                                 bias=be_sb[:, 1:2])
            for c in range(MPC):
                nc.tensor.matmul(s_ps[c][:], fc1_sb[:, q * 128:(q + 1) * 128],
                                 e1[:, c * L:(c + 1) * L],
                                 start=(q == 0), stop=False)
            for c in range(MPC):
                nc.tensor.matmul(s_ps[c][:], fc2_sb[:, q * 128:(q + 1) * 128],
                                 e2[:, c * L:(c + 1) * L],
                                 start=False, stop=(q == QF - 1))

        # ---- W = tanh(s + b2)  [128 slots, (m,l)] ----
        W_sb = cpool.tile([128, CL], FP, tag="W")
        for c in range(MPC):
            nc.scalar.activation(W_sb[:, c * L:(c + 1) * L], s_ps[c][:],
                                 AF.Tanh, bias=b2_sb[:, 0:1])
        if padded:
            nc.vector.tensor_scalar_add(W_sb[:], W_sb[:], pad_sb[:, 0:1])

        # ---- atom-side: Wc = exp(5*max_l W), masked segment sums ----
        wcp_sb = spool.tile([128, 1], FP, tag="wcp")
        for m in range(MPC):
            nc.vector.reduce_max(wcp_sb[32 * m:32 * (m + 1), :],
                                 W_sb[32 * m:32 * (m + 1), m * L:(m + 1) * L],
                                 axis=mybir.AxisListType.X)
        wc_sb = spool.tile([128, 1], FP, tag="wc")
        nc.scalar.activation(wc_sb[:], wcp_sb[:], AF.Exp, scale=5.0)
        wc4_sb = spool.tile([128, MPC], FP, tag="wc4")
        nc.vector.tensor_mul(wc4_sb[:], seg_sb[:],
                             wc_sb[:, 0:1].to_broadcast([128, MPC]))
        wc4_bf = spool.tile([128, MPC], BF, tag="wc4bf")
        nc.vector.tensor_copy(wc4_bf[:], wc4_sb[:])
        sc_ps = pp_mm.tile([1, MPC], FP, tag="mm")
        nc.tensor.matmul(sc_ps[:], onec_sb[:], wc4_bf[:])
        ap_ps = pp_mm.tile([128, MPC], FP, tag="mm")
        nc.tensor.matmul(ap_ps[:], atomN_sb[:], wc4_sb[:])

        # ---- prot-side: WpT[l, (c,m)] via transposes + segment max ----
        WpT_sb = cpool.tile([128, 4 * MPC], FP, tag="WpT")
        for m in range(MPC):
            for c in range(4):
                tp = pp_pa.tile([128, 128], FP, tag="pa")
                nc.tensor.transpose(
                    tp[:], W_sb[:, m * L + c * 128:m * L + (c + 1) * 128],
                    id_sb[:])
                nc.vector.reduce_max(WpT_sb[:, c * MPC + m:c * MPC + m + 1],
                                     tp[:, 32 * m:32 * (m + 1)],
                                     axis=mybir.AxisListType.X)

        expW_sb = spool.tile([128, 4 * MPC], BF, tag="expW")
        nc.scalar.activation(expW_sb[:], WpT_sb[:], AF.Exp, scale=5.0)

        den_ps = pp_mm.tile([1, 4 * MPC], FP, tag="mm")
        nc.tensor.matmul(den_ps[:], onec_sb[:], expW_sb[:])
        nrm = spool.tile([1, 2 * MPC], FP, tag="nrm")
        nc.vector.tensor_copy(nrm[:, 0:MPC], sc_ps[:])
        nc.vector.reduce_sum(nrm[:, MPC:2 * MPC],
                             den_ps[:].rearrange("p (c m) -> p m c", m=MPC),
                             axis=mybir.AxisListType.X)
        rnrm = spool.tile([1, 2 * MPC], FP, tag="rnrm")
        nc.vector.reciprocal(rnrm[:], nrm[:])
        rb_ps = pp_mm.tile([128, 2 * MPC], FP, tag="mm")
        nc.tensor.matmul(rb_ps[:], oner_sb[:], rnrm[:])
        rb_sb = spool.tile([128, 2 * MPC], FP, tag="rb")
        nc.vector.tensor_copy(rb_sb[:], rb_ps[:])

        apT_sb = spool.tile([128, MPC], FP, tag="apT")
        nc.vector.tensor_mul(apT_sb[:], ap_ps[:], rb_sb[:, 0:MPC])
        pp_ps = pp_mm.tile([128, MPC], FP, tag="mm")
        for m in range(MPC):
            for c in range(4):
                nc.tensor.matmul(pp_ps[:, m:m + 1],
                                 protN_sb[:, (m * 4 + c) * 128:(m * 4 + c + 1) * 128],
                                 expW_sb[:, c * MPC + m:c * MPC + m + 1],
                                 start=(c == 0), stop=(c == 3))
        ppT_sb = spool.tile([128, MPC], FP, tag="ppT")
        nc.vector.tensor_mul(ppT_sb[:], pp_ps[:], rb_sb[:, MPC:2 * MPC])

        # ---- output MLP (molecules on the free axis) ----
        h1_sb = spool.tile([128, NJ1 * MPC], FP, tag="h1")
        for j in range(NJ1):
            h1_ps = pp_mm.tile([128, MPC], FP, tag="mm")
            nc.tensor.matmul(h1_ps[:], d1_sb[:, j * 128:(j + 1) * 128],
                             apT_sb[:], start=True, stop=False)
            nc.tensor.matmul(h1_ps[:], d1_sb[:, H1 + j * 128:H1 + (j + 1) * 128],
                             ppT_sb[:], start=False, stop=True)
            nc.scalar.activation(h1_sb[:, j * MPC:(j + 1) * MPC], h1_ps[:],
                                 AF.Relu, bias=d1b_sb[:, j:j + 1])
        h2_sb = spool.tile([128, NJ2 * MPC], FP, tag="h2")
        for i in range(NJ2):
            h2_ps = pp_mm.tile([128, MPC], FP, tag="mm")
            for j in range(NJ1):
                nc.tensor.matmul(h2_ps[:],
                                 d2_sb[:, j * H2 + i * 128:j * H2 + (i + 1) * 128],
                                 h1_sb[:, j * MPC:(j + 1) * MPC],
                                 start=(j == 0), stop=(j == NJ1 - 1))
            nc.scalar.activation(h2_sb[:, i * MPC:(i + 1) * MPC], h2_ps[:],
                                 AF.Relu, bias=d2b_sb[:, i:i + 1])
        o_ps = pp_mm.tile([1, MPC], FP, tag="mm")
        for i in range(NJ2):
            nc.tensor.matmul(o_ps[:], ow_sb[:, i:i + 1],
                             h2_sb[:, i * MPC:(i + 1) * MPC],
                             start=(i == 0), stop=(i == NJ2 - 1))
        o_sb = spool.tile([1, MPC], FP, tag="o")
        nc.scalar.activation(o_sb[:], o_ps[:], AF.Identity, bias=ob_sb[0:1, 0:1])
        nc.sync.dma_start(out_d[:], o_sb[0:1, :])

    nc.compile()
    return nc


def prepare(atom_embed, protSeq_embed, atom_splits,
            att1_W, att1_b, att2_W, att2_b,
            d1_W, d1_b, d2_W, d2_b, out_W, out_b):
    atom_embed = np.ascontiguousarray(atom_embed, dtype=np.float32)
    protSeq_embed = np.ascontiguousarray(protSeq_embed, dtype=np.float32)
    splits = np.asarray(atom_splits).astype(np.int64)
    assert atom_embed.shape == (N, A) and protSeq_embed.shape == (B, L, P)

    counts = np.bincount(splits, minlength=B)
    starts = np.concatenate([[0], np.cumsum(counts)])[:B]
    assert counts.max() <= 32, "fast path requires <=32 atoms per molecule"
    cap = 32
    padded = bool((counts != cap).any())

    key = ("sin", padded)
    if key not in _cache:
        _cache[key] = _build(padded)
    nc = _cache[key]

    import ml_dtypes
    f32 = np.float32
    w2 = np.asarray(att2_W, f32)[:, 0]
    fcs = np.stack([a * w2 for a in ACOEF], axis=1).astype(f32)
    biasE = np.stack([np.full(128, B5, f32), np.full(128, B3, f32)], axis=1)
    shared = {
        "att1_Wp": np.ascontiguousarray(att1_W[:P], f32).astype(ml_dtypes.bfloat16),
        "att1_Wa": np.ascontiguousarray(att1_W[P:], f32),
        "b1": np.asarray(att1_b, f32).reshape(128, 1),
        "fcs": fcs,
        "b2vec": np.full((128, 1), np.asarray(att2_b, f32)[0], f32),
        "biasE": biasE,
        "ident": np.eye(128, dtype=f32),
        "d1_W": np.ascontiguousarray(d1_W, f32),
        "d1_b4": np.ascontiguousarray(np.asarray(d1_b, f32).reshape(H1 // 128, 128).T),
        "d2_W": np.ascontiguousarray(d2_W, f32),
        "d2_b2": np.ascontiguousarray(np.asarray(d2_b, f32).reshape(H2 // 128, 128).T),
        "oW": np.asarray(out_W, f32).reshape(H2, 1),
        "ob": np.asarray(out_b, f32).reshape(1, 1),
        "ones_c": np.ones((128, 1), ml_dtypes.bfloat16),
        "ones_r": np.ones((1, 128), f32),
    }

    in_maps = []
    for cidx in range(NCORES):
        gm = range(MPC * cidx, MPC * (cidx + 1))
        aN = np.zeros((128, A), f32)
        seg = np.zeros((128, MPC), f32)
        pad = np.full((128, 1), PAD_NEG, f32)
        for lm, g in enumerate(gm):
            cnt = int(counts[g])
            s0 = lm * cap
            aN[s0:s0 + cnt] = atom_embed[starts[g]:starts[g] + cnt]
            seg[s0:s0 + cnt, lm] = 1.0
            pad[s0:s0 + cnt, 0] = 0.0
        pmc = protSeq_embed[MPC * cidx:MPC * (cidx + 1)]
        in_maps.append({
            **shared,
            "prot_T": np.ascontiguousarray(pmc.transpose(0, 2, 1)).astype(ml_dtypes.bfloat16),
            "prot_N": np.ascontiguousarray(pmc).astype(ml_dtypes.bfloat16),
            "atom_N": aN,
            "atom_T": np.ascontiguousarray(aN.T),
            "seg_m": seg,
            "pad_add": pad,
        })

    return nc, in_maps


def kernel(**inputs):
    nc, in_maps = prepare(**inputs)
    res = run_bass_kernel_spmd(nc, in_maps, list(range(NCORES)))
    return np.concatenate([res.results[c]["out"] for c in range(NCORES)], axis=0)
